# revision 1
# baseline (speedup 1.0000x reference)
"""DigitCaps dynamic-routing kernel for 8 Trainium2 NeuronCores.

Math (reference):
  u_hat[b,i,j,d] = sum_e W[0,i,j,d,e] * x[b,i,e]
  2 routing iterations; iteration 1 has b=0 so c = 1/32 exactly:
    s1 = (1/32) sum_i u_hat ;  v1 = squash(s1)
    b2 = sum_d u_hat * v1    ;  c2 = softmax_j(b2)
    s2 = sum_i c2 * u_hat    ;  v2 = squash(s2)   -> output

Sharding: in_capsules (i) split 8 ways (256 per core); batch b=128 lives in
the partition dimension everywhere.  The two reductions over i (s1, s2) are
[128,512] fp32 AllReduces.  Softmax/b2 are i-local, so no other comms.

Per-core layouts (host-prepped, zero math on host — pure transpose):
  wt[r, e, g, jd] : [4, 8, 64, 512]  W for i = core*256 + g*4 + r
  xt[r, e, g, b]  : [4, 8, 64, 128]  x likewise
On SBUF each strip r sits at partition base 32*r (rows e=0..7 used), so the
K=8 matmuls auto-derive tile_position=(32r,0) and run 4-way row-tiled.
"""

import sys
for _p in ("/opt/pypackages", "/opt/trn_rl_repo"):
    if _p not in sys.path:
        sys.path.insert(0, _p)

import numpy as np

import concourse.bass as bass
import concourse.bacc as bacc
import concourse.tile as tile
from concourse import mybir
from concourse.bass_utils import run_bass_kernel_spmd

B = 128
I = 2048
E = 8
J = 32
D = 16
JD = J * D          # 512
NC_ = 8             # cores
IS = I // NC_       # 256 in_caps per core
NG = IS // 4        # 64 groups of 4 strips
WTILE_G = 8         # groups per streamed W tile
EPS = 1e-8

f32 = mybir.dt.float32
f32r = mybir.dt.float32r
bf16 = mybir.dt.bfloat16


def _bc(ap, n):
    """Broadcast an AP along a new innermost dim of size n (step 0)."""
    return bass.AP(tensor=ap.tensor, offset=ap.offset, ap=[*ap.ap, [0, n]])


def _squash(nc, pool, s_sb, v_sb):
    """v = |s|^2/(1+|s|^2) * s/(|s|+eps), norm over d within each j."""
    sq = pool.tile([B, JD], f32, tag="sqs")
    nc.vector.tensor_mul(sq[:, :], s_sb[:, :], s_sb[:, :])
    n2 = pool.tile([B, J], f32, tag="sqn2")
    nc.vector.tensor_reduce(
        out=n2[:, :], in_=sq[:, :].rearrange("p (j d) -> p j d", d=D),
        axis=mybir.AxisListType.X, op=mybir.AluOpType.add)
    nrm = pool.tile([B, J], f32, tag="sqn")
    nc.scalar.sqrt(out=nrm[:, :], in_=n2[:, :])
    a1 = pool.tile([B, J], f32, tag="sqa")
    nc.vector.tensor_scalar_add(a1[:, :], n2[:, :], 1.0)
    b1 = pool.tile([B, J], f32, tag="sqb")
    nc.vector.tensor_scalar_add(b1[:, :], nrm[:, :], EPS)
    den = pool.tile([B, J], f32, tag="sqden")
    nc.vector.tensor_mul(den[:, :], a1[:, :], b1[:, :])
    rden = pool.tile([B, J], f32, tag="sqrden")
    nc.vector.reciprocal(out=rden[:, :], in_=den[:, :])
    sc = pool.tile([B, J], f32, tag="sqsc")
    nc.vector.tensor_mul(sc[:, :], n2[:, :], rden[:, :])
    nc.vector.tensor_tensor(
        out=v_sb[:, :].rearrange("p (j d) -> p j d", d=D),
        in0=s_sb[:, :].rearrange("p (j d) -> p j d", d=D),
        in1=_bc(sc[:, :], D), op=mybir.AluOpType.mult)


def build_nc(num_devices=NC_, with_cc=True):
    nc = bacc.Bacc("TRN2", target_bir_lowering=False, debug=False,
                   num_devices=num_devices)
    wt = nc.dram_tensor("wt", [4, E, NG, JD], f32r, kind="ExternalInput")
    xt = nc.dram_tensor("xt", [4, E, NG, B], f32r, kind="ExternalInput")
    yout = nc.dram_tensor("yout", [B, JD], f32, kind="ExternalOutput")

    cc1_in = nc.dram_tensor("cc1_in", [B, JD], f32)
    cc1_out = nc.dram_tensor("cc1_out", [B, JD], f32, addr_space="Shared")
    cc2_in = nc.dram_tensor("cc2_in", [B, JD], f32)
    cc2_out = nc.dram_tensor("cc2_out", [B, JD], f32, addr_space="Shared")
    rgroups = [list(range(num_devices))]

    def allreduce(cin, cout):
        if with_cc:
            nc.gpsimd.collective_compute(
                "AllReduce", mybir.AluOpType.add, replica_groups=rgroups,
                ins=[cin.ap()], outs=[cout.ap()])
        else:
            nc.sync.dma_start(out=cout.ap(), in_=cin.ap())

    with tile.TileContext(nc) as tc:
        with (
            tc.tile_pool(name="consts", bufs=1) as consts,
            tc.tile_pool(name="wpool", bufs=2) as wpool,
            tc.tile_pool(name="mpool", bufs=2) as mpool,
            tc.tile_pool(name="spool", bufs=2) as spool,
        ):
            # ---- load x (stationary) : strips at partition base 32r ----
            xg = consts.tile([128, NG, B], f32r, tag="xg")
            for r in range(4):
                nc.sync.dma_start(out=xg[32 * r:32 * r + E, :, :],
                                  in_=xt[r, :, :, :])

            # ---- pass 1: s1_part = sum_i u_hat_i  (PSUM accumulation) ----
            p1_cm = tc.tile_pool(name="p1", bufs=1, space="PSUM")
            p1 = p1_cm.__enter__()
            s1p = [p1.tile([B, JD], f32, tag=f"s1p{r}", name=f"s1p{r}")
                   for r in range(4)]
            for wti in range(NG // WTILE_G):
                wtile = wpool.tile([128, WTILE_G, JD], f32r, tag="wt")
                for r in range(4):
                    nc.sync.dma_start(
                        out=wtile[32 * r:32 * r + E, :, :],
                        in_=wt[r, :, wti * WTILE_G:(wti + 1) * WTILE_G, :])
                for gg in range(WTILE_G):
                    g = wti * WTILE_G + gg
                    for r in range(4):
                        nc.tensor.matmul(
                            out=s1p[r][:, :],
                            lhsT=xg[32 * r:32 * r + E, g, :],
                            rhs=wtile[32 * r:32 * r + E, gg, :],
                            start=(g == 0), stop=(g == NG - 1),
                            tile_position=(32 * r, 0))
            # combine 4 strip-partials (one PSUM operand per DVE op)
            t01 = spool.tile([B, JD], f32, tag="t01")
            ssum = consts.tile([B, JD], f32, tag="ssum")
            nc.vector.tensor_copy(t01[:, :], s1p[0][:, :])
            nc.vector.tensor_add(t01[:, :], t01[:, :], s1p[1][:, :])
            nc.vector.tensor_add(t01[:, :], t01[:, :], s1p[2][:, :])
            nc.vector.tensor_add(ssum[:, :], t01[:, :], s1p[3][:, :])
            p1_cm.__exit__(None, None, None)

            # ---- AllReduce s1 over 8 cores ----
            nc.sync.dma_start(out=cc1_in.ap(), in_=ssum[:, :])
            allreduce(cc1_in, cc1_out)
            s1 = consts.tile([B, JD], f32, tag="s1")
            nc.sync.dma_start(out=s1[:, :], in_=cc1_out.ap())
            nc.vector.tensor_scalar_mul(s1[:, :], s1[:, :], 1.0 / J)

            # ---- v1 = squash(s1) ----
            v1 = consts.tile([B, JD], f32, tag="v1")
            _squash(nc, spool, s1, v1)
            # bf16 v1 replicated over the 4 strips (enables 2x DVE mode)
            v1x4 = consts.tile([B, 4, JD], bf16, tag="v1x4")
            for r in range(4):
                nc.vector.tensor_copy(v1x4[:, r, :], v1[:, :])

            # ---- pass 2: routing iteration 2, i-local ----
            # Per group of 4 capsules: one [B, 4*JD] fused op per stage.
            up_cm = tc.tile_pool(name="up", bufs=2, space="PSUM")
            up = up_cm.__enter__()
            s2a = [consts.tile([B, 4, JD], f32, tag=f"s2a{h}",
                               name=f"s2a{h}") for h in range(2)]
            nc.gpsimd.memset(s2a[0][:, :, :], 0.0)
            nc.vector.memset(s2a[1][:, :, :], 0.0)
            for wti in range(NG // WTILE_G):
                wtile = wpool.tile([128, WTILE_G, JD], f32r, tag="wt")
                for r in range(4):
                    nc.sync.dma_start(
                        out=wtile[32 * r:32 * r + E, :, :],
                        in_=wt[r, :, wti * WTILE_G:(wti + 1) * WTILE_G, :])
                for gg in range(WTILE_G):
                    g = wti * WTILE_G + gg
                    # u_hat for the 4 strips: one PSUM tile, 4 row-tiled MMs
                    u4 = up.tile([B, 4, JD], f32, tag="u4")
                    for r in range(4):
                        nc.tensor.matmul(
                            out=u4[:, r, :],
                            lhsT=xg[32 * r:32 * r + E, g, :],
                            rhs=wtile[32 * r:32 * r + E, gg, :],
                            start=True, stop=True,
                            tile_position=(32 * r, 0))
                    # bf16 copy of u_hat (ACT) -> 2x DVE mode downstream
                    usb = mpool.tile([B, 4, JD], bf16, tag="usb", bufs=4)
                    nc.scalar.copy(out=usb[:, :, :], in_=u4[:, :, :])
                    # m = u*v1 ; b2[b, (r j)] = sum_d m
                    m = mpool.tile([B, 4, JD], bf16, tag="m")
                    nc.vector.tensor_mul(m[:, :, :], usb[:, :, :],
                                         v1x4[:, :, :])
                    b2g = spool.tile([B, 4 * J], f32, tag="b2g")
                    nc.vector.tensor_reduce(
                        out=b2g[:, :],
                        in_=m[:, :, :].rearrange("p r (j d) -> p (r j) d",
                                                 d=D),
                        axis=mybir.AxisListType.X, op=mybir.AluOpType.add)
                    # softmax over j for the 4 capsules at once
                    # exp + per-capsule sum fused on ACT (accum_out);
                    # frees the DVE z-reduce
                    eg = spool.tile([B, 4 * J], f32, tag="eg")
                    z = spool.tile([B, 4], f32, tag="z")
                    for r in range(4):
                        nc.scalar.activation(
                            out=eg[:, r * J:(r + 1) * J],
                            in_=b2g[:, r * J:(r + 1) * J],
                            func=mybir.ActivationFunctionType.Exp,
                            accum_out=z[:, r:r + 1])
                    rz = spool.tile([B, 4], f32, tag="rz")
                    nc.vector.reciprocal(out=rz[:, :], in_=z[:, :])
                    c2g = spool.tile([B, 4, J], f32, tag="c2g")
                    nc.gpsimd.tensor_tensor(
                        out=c2g[:, :, :],
                        in0=eg[:, :].rearrange("p (r j) -> p r j", j=J),
                        in1=_bc(rz[:, :], J), op=mybir.AluOpType.mult)
                    # s2 += c2 * u_hat : split groups DVE / GPSIMD (~40/60,
                    # GPSIMD is ~2x slower per op but otherwise idle)
                    h = 0 if (g % 2) == 0 else 1
                    eng = nc.vector if h == 0 else nc.gpsimd
                    t = mpool.tile([B, 4, J, D], f32, tag=f"t{h}",
                                   name=f"t{h}")
                    eng.tensor_tensor(
                        out=t[:, :, :, :],
                        in0=usb[:, :, :].rearrange("p r (j d) -> p r j d",
                                                   d=D),
                        in1=_bc(c2g[:, :, :], D), op=mybir.AluOpType.mult)
                    eng.tensor_add(
                        s2a[h][:, :, :],
                        s2a[h][:, :, :],
                        t[:, :, :, :].rearrange("p r j d -> p r (j d)"))
            # fold the two [B, 4, JD] accumulators down to [B, JD]
            s2r = [spool.tile([B, JD], f32, tag=f"s2r{h}", name=f"s2r{h}")
                   for h in range(2)]
            for h in range(2):
                nc.vector.tensor_reduce(
                    out=s2r[h][:, :],
                    in_=s2a[h][:, :, :].rearrange("p r c -> p c r"),
                    axis=mybir.AxisListType.X, op=mybir.AluOpType.add)
            s2part = consts.tile([B, JD], f32, tag="s2part")
            nc.vector.tensor_add(s2part[:, :], s2r[0][:, :], s2r[1][:, :])
            up_cm.__exit__(None, None, None)

            # ---- AllReduce s2 ----
            nc.sync.dma_start(out=cc2_in.ap(), in_=s2part[:, :])
            allreduce(cc2_in, cc2_out)
            s2 = consts.tile([B, JD], f32, tag="s2")
            nc.sync.dma_start(out=s2[:, :], in_=cc2_out.ap())

            # ---- v2 = squash(s2) -> output ----
            v2 = consts.tile([B, JD], f32, tag="v2")
            _squash(nc, spool, s2, v2)
            nc.sync.dma_start(out=yout.ap(), in_=v2[:, :])

    nc.compile()
    return nc


_NC_CACHE = None


def _get_nc():
    global _NC_CACHE
    if _NC_CACHE is None:
        _NC_CACHE = build_nc()
    return _NC_CACHE


def _prep_inputs(x, W):
    """Pure layout transform: returns per-core wt, xt arrays."""
    # W[0]: [I, J, D, E] -> [core, g, r, j, d, e] -> [core, r, e, g, (j d)]
    Wv = np.ascontiguousarray(W[0]).reshape(NC_, NG, 4, J, D, E)
    wt = np.ascontiguousarray(Wv.transpose(0, 2, 5, 1, 3, 4)).reshape(
        NC_, 4, E, NG, JD)
    # x: [B, I, E] -> [i, e, b] -> [core, g, r, e, b] -> [core, r, e, g, b]
    xv = np.ascontiguousarray(x.transpose(1, 2, 0)).reshape(
        NC_, NG, 4, E, B)
    xt = np.ascontiguousarray(xv.transpose(0, 2, 3, 1, 4))
    return wt, xt


def run(x, W, trace=False, **kw):
    x = np.asarray(x, dtype=np.float32)
    W = np.asarray(W, dtype=np.float32)
    wt, xt = _prep_inputs(x, W)
    nc = _get_nc()
    in_maps = [{"wt": wt[c], "xt": xt[c]} for c in range(NC_)]
    res = run_bass_kernel_spmd(nc, in_maps, core_ids=list(range(NC_)),
                               trace=trace, **kw)
    out = res.results[0]["yout"].reshape(B, J, D).astype(np.float32)
    return out, res


def kernel(x, W):
    out, _ = run(x, W)
    return out



# revision 9
# speedup vs baseline: 2.2185x; 2.2185x over previous
"""DigitCaps dynamic-routing kernel for 8 Trainium2 NeuronCores — v2.

Math (reference):
  u_hat[b,i,j,d] = sum_e W[0,i,j,d,e] * x[b,i,e]
  2 routing iterations; iteration 1 has b=0 so c = 1/32 exactly:
    s1 = (1/32) sum_i u_hat ;  v1 = squash(s1)
    b2 = sum_d u_hat * v1    ;  c2 = softmax_j(b2)
    s2 = sum_i c2 * u_hat    ;  v2 = squash(s2)   -> output

Structure (per core, i sharded 8 ways, batch b=128 in partitions):
  s1: ONE K=2048 GEMM  s1[b,(j,d)] = sum_{(e,i)} xT[(e,i),b] * W1[(e,i),(j,d)]
  pass 2 avoids u_hat entirely:
    A[b,j,(i,e)] = sum_d v1[b,j,d] W[i,j,d,e]      (per-j GEMM, contract d)
    b2[b,i,j]    = sum_e x[b,i,e] A[b,j,i,e]       (DVE mult + pairwise tree)
    e_expT       = exp(b2^T)  (PE transpose, exp fused into PSUM copy-out)
    zT,rzT       = softmax denom in transposed land (pairwise adds over j)
    yT[(e,i),j,b]= (xT*rzT) * e_expT               (DVE broadcast mult)
    s2[b,(j,d)]  = sum_{(e,i)} yT * W              (per-(j,kt) GEMM, N=16)
  Two [128,512] fp32 AllReduces (s1, s2); softmax/b2 are i-local.
"""

import sys
for _p in ("/opt/pypackages", "/opt/trn_rl_repo"):
    if _p not in sys.path:
        sys.path.insert(0, _p)

import numpy as np
import ml_dtypes

import concourse.bass as bass
import concourse.bacc as bacc
import concourse.tile as tile
from concourse import mybir
from concourse.bass_utils import run_bass_kernel_spmd

B = 128
I = 2048
E = 8
J = 32
D = 16
JD = J * D          # 512
NC_ = 8             # cores
IS = I // NC_       # 256 in_caps per core
EPS = 1e-8

f32 = mybir.dt.float32
bf16 = mybir.dt.bfloat16

# ---- engine assignment knobs (tuned against TimelineSim) ----
# 64 chunks (h-major: chunk = h*32 + j): who does the A*x mult
#   'act' = ACT copies PSUM->bf16 then DVE mult; 'dve' = DVE direct from
#   PSUM.  (GPSIMD cannot read PSUM on hardware.)
P_PATH = ['act' if (c % 8) < 5 else 'dve' for c in range(64)]
# who does the 3-round pairwise e-reduce per chunk
RED_ENG = ['dve' if (c % 8) < 5 else 'gps' for c in range(64)]
# yT mult halves (32 = 16 kt x 2 jhalf)
YT_ENG = ['gps' if (c % 8) == 7 else 'dve' for c in range(32)]


def _bc(ap, n):
    """Broadcast an AP along a new innermost dim of size n (step 0)."""
    return bass.AP(tensor=ap.tensor, offset=ap.offset, ap=[*ap.ap, [0, n]])


def _bc_mid(ap, n):
    """Broadcast an AP along a new dim of size n inserted before the last
    free dim (step 0)."""
    return bass.AP(tensor=ap.tensor, offset=ap.offset,
                   ap=[*ap.ap[:-1], [0, n], ap.ap[-1]])


def _strided(ap, dims):
    """Replace the free dims of a [P, 1]-sliced AP with custom [step, num]
    pairs (partition dim kept)."""
    return bass.AP(tensor=ap.tensor, offset=ap.offset, ap=[ap.ap[0], *dims])


def _squash(nc, pool, s_sb, v_sb):
    """v = |s|^2/(1+|s|^2) * s/(|s|+eps), norm over d within each j."""
    sq = pool.tile([B, JD], f32, tag="sqs")
    nc.vector.tensor_mul(sq[:, :], s_sb[:, :], s_sb[:, :])
    n2 = pool.tile([B, J], f32, tag="sqn2")
    nc.vector.tensor_reduce(
        out=n2[:, :], in_=sq[:, :].rearrange("p (j d) -> p j d", d=D),
        axis=mybir.AxisListType.X, op=mybir.AluOpType.add)
    nrm = pool.tile([B, J], f32, tag="sqn")
    nc.scalar.sqrt(out=nrm[:, :], in_=n2[:, :])
    a1 = pool.tile([B, J], f32, tag="sqa")
    nc.vector.tensor_scalar_add(a1[:, :], n2[:, :], 1.0)
    b1 = pool.tile([B, J], f32, tag="sqb")
    nc.vector.tensor_scalar_add(b1[:, :], nrm[:, :], EPS)
    den = pool.tile([B, J], f32, tag="sqden")
    nc.vector.tensor_mul(den[:, :], a1[:, :], b1[:, :])
    rden = pool.tile([B, J], f32, tag="sqrden")
    nc.vector.reciprocal(out=rden[:, :], in_=den[:, :])
    sc = pool.tile([B, J], f32, tag="sqsc")
    nc.vector.tensor_mul(sc[:, :], n2[:, :], rden[:, :])
    nc.vector.tensor_tensor(
        out=v_sb[:, :].rearrange("p (j d) -> p j d", d=D),
        in0=s_sb[:, :].rearrange("p (j d) -> p j d", d=D),
        in1=_bc(sc[:, :], D), op=mybir.AluOpType.mult)


def build_nc(num_devices=NC_, with_cc=True):
    nc = bacc.Bacc("TRN2", target_bir_lowering=False, debug=False,
                   num_devices=num_devices)
    # host-prepped per-core inputs (see _prep_inputs for layouts)
    xt_d = nc.dram_tensor("xt", [128, 16, B], bf16, kind="ExternalInput")
    xb_d = nc.dram_tensor("xb", [B, IS * E], bf16, kind="ExternalInput")
    w1_d = nc.dram_tensor("w1", [128, 16, JD], bf16, kind="ExternalInput")
    wa_d = nc.dram_tensor("wa", [128, 8, IS * E], bf16, kind="ExternalInput")
    ws_d = nc.dram_tensor("ws", [128, J, 16, D], bf16, kind="ExternalInput")
    id_d = nc.dram_tensor("ident", [128, 128], f32, kind="ExternalInput")
    yout = nc.dram_tensor("yout", [B, JD], f32, kind="ExternalOutput")

    cc1_in = nc.dram_tensor("cc1_in", [B, JD], f32)
    cc1_out = nc.dram_tensor("cc1_out", [B, JD], f32, addr_space="Shared")
    cc2_in = nc.dram_tensor("cc2_in", [B, JD], f32)
    cc2_out = nc.dram_tensor("cc2_out", [B, JD], f32, addr_space="Shared")
    rgroups = [list(range(num_devices))]

    def allreduce(cin, cout):
        if with_cc:
            nc.gpsimd.collective_compute(
                "AllReduce", mybir.AluOpType.add, replica_groups=rgroups,
                ins=[cin.ap()], outs=[cout.ap()])
        else:
            nc.sync.dma_start(out=cout.ap(), in_=cin.ap())

    with tile.TileContext(nc) as tc:
        with (
            tc.tile_pool(name="consts", bufs=1) as consts,
            tc.tile_pool(name="work", bufs=1) as work,
            tc.tile_pool(name="ppool", bufs=2) as ppool,
            tc.tile_pool(name="ypool", bufs=2) as ypool,
            tc.tile_pool(name="pa", bufs=2, space="PSUM") as pa,
            tc.tile_pool(name="ptp", bufs=1, space="PSUM") as ptp,
            tc.tile_pool(name="pacc", bufs=1, space="PSUM") as pacc,
        ):
            # ---------- input DMAs ----------
            ident = consts.tile([128, 128], f32, tag="ident")
            nc.sync.dma_start(out=ident[:, :], in_=id_d.ap())
            xt = consts.tile([128, 16, B], bf16, tag="xt")
            nc.sync.dma_start(out=xt[:, :, :], in_=xt_d.ap())
            w1 = consts.tile([128, 16, JD], bf16, tag="w1")
            for q in range(4):
                nc.sync.dma_start(out=w1[:, 4 * q:4 * q + 4, :],
                                  in_=w1_d.ap()[:, 4 * q:4 * q + 4, :])
            xb = consts.tile([B, IS * E], bf16, tag="xb")
            nc.sync.dma_start(out=xb[:, :], in_=xb_d.ap())
            wa = consts.tile([128, 8, IS * E], bf16, tag="wa")
            for q in range(4):
                nc.sync.dma_start(out=wa[:, 2 * q:2 * q + 2, :],
                                  in_=wa_d.ap()[:, 2 * q:2 * q + 2, :])
            ws = consts.tile([128, J, 16, D], bf16, tag="ws")
            nc.sync.dma_start(out=ws[:, :, :, :], in_=ws_d.ap())

            # ---------- pass 1: s1 GEMM (w1 pre-scaled by 1/32) ----------
            ps1 = pacc.tile([B, JD], f32, tag="ps1", name="ps1")
            for kt in range(16):
                nc.tensor.matmul(out=ps1[:, :], lhsT=xt[:, kt, :],
                                 rhs=w1[:, kt, :],
                                 start=(kt == 0), stop=(kt == 15))
            s1c = work.tile([B, JD], f32, tag="s1c")
            nc.vector.tensor_copy(s1c[:, :], ps1[:, :])
            nc.sync.dma_start(out=cc1_in.ap(), in_=s1c[:, :])
            allreduce(cc1_in, cc1_out)
            s1 = consts.tile([B, JD], f32, tag="s1")
            nc.sync.dma_start(out=s1[:, :], in_=cc1_out.ap())

            # ---------- v1 = squash(s1) ----------
            v1 = consts.tile([B, JD], f32, tag="v1")
            _squash(nc, work, s1, v1)

            # ---------- v1T (padded 32-row slots, 4 j per tile) ----------
            # v1pad[b, t, 32s+r] = v1[b, 64t+16s+r] for r<16, 0 for r>=16;
            # after transpose: v1t[32*(j%4)+d, j//4, b] = v1[b, 16j+d]
            v1pad = consts.tile([B, 8, 128], f32, tag="v1pad")
            nc.vector.memset(v1pad[:, :, :], 0.0)
            for s in range(4):
                nc.vector.tensor_copy(
                    _strided(v1pad[:, 0, 32 * s:32 * s + 1], [[128, 8], [1, 16]]),
                    _strided(v1[:, 16 * s:16 * s + 1], [[64, 8], [1, 16]]))
            v1t = consts.tile([128, 8, B], bf16, tag="v1t")
            tph = [ptp.tile([128, 512], f32, tag=f"tph{h}", name=f"tph{h}")
                   for h in range(2)]
            for grp in range(2):
                tp = tph[grp]
                for tt in range(4):
                    t = grp * 4 + tt
                    nc.tensor.transpose(out=tp[:, 128 * tt:128 * tt + 128],
                                        in_=v1pad[:, t, :],
                                        identity=ident[:, :])
                nc.scalar.copy(out=v1t[:, 4 * grp:4 * grp + 4, :], in_=tp[:, :])

            # ---------- pass 2a: per-(h,j) A-GEMM -> b2 -> e_expT --------
            b2 = consts.tile([B, 2, J, 128], f32, tag="b2")
            eeT = [consts.tile([128, J, B], bf16, tag=f"eeT{h}",
                               name=f"eeT{h}") for h in range(2)]
            for h in range(2):
                for j in range(J):
                    chunk = h * J + j
                    s, t = j % 4, j // 4
                    A = pa.tile([B, 1024], f32, tag="A")
                    for iq in range(2):
                        off = 1024 * h + 512 * iq
                        nc.tensor.matmul(
                            out=A[:, 512 * iq:512 * iq + 512],
                            lhsT=v1t[32 * s:32 * s + 32, t, :],
                            rhs=wa[32 * s:32 * s + 32, t, off:off + 512],
                            start=True, stop=True, tile_position=(32 * s, 0))
                    xs = xb[:, 1024 * h:1024 * h + 1024]
                    P = ppool.tile([B, 1024], bf16, tag="P")
                    path = P_PATH[chunk]
                    if path == 'act':
                        Ac = ppool.tile([B, 1024], bf16, tag="Ac")
                        nc.scalar.copy(out=Ac[:, :], in_=A[:, :])
                        nc.vector.tensor_mul(P[:, :], Ac[:, :], xs)
                    elif path == 'dve':
                        nc.vector.tensor_mul(P[:, :], A[:, :], xs)
                    else:
                        nc.gpsimd.tensor_mul(P[:, :], A[:, :], xs)
                    # pairwise tree over e=8 (i128 x e8 per half)
                    eng = nc.vector if RED_ENG[chunk] == 'dve' else nc.gpsimd
                    Pv = P[:, :].rearrange("p (i e) -> p i e", e=8)
                    r1 = ppool.tile([B, 512], bf16, tag="r1")
                    eng.tensor_tensor(
                        out=r1[:, :].rearrange("p (i e) -> p i e", e=4),
                        in0=Pv[:, :, 0:4], in1=Pv[:, :, 4:8],
                        op=mybir.AluOpType.add)
                    r1v = r1[:, :].rearrange("p (i e) -> p i e", e=4)
                    r2 = ppool.tile([B, 256], bf16, tag="r2")
                    eng.tensor_tensor(
                        out=r2[:, :].rearrange("p (i e) -> p i e", e=2),
                        in0=r1v[:, :, 0:2], in1=r1v[:, :, 2:4],
                        op=mybir.AluOpType.add)
                    eng.tensor_tensor(
                        out=b2[:, h, j, :],
                        in0=_strided(r2[:, 0:1], [[2, 128]]),
                        in1=_strided(r2[:, 1:2], [[2, 128]]),
                        op=mybir.AluOpType.add)
                    # transpose + (after groups of 4 j) exp copy-out
                    nc.tensor.transpose(
                        out=tph[h][:, 128 * (j % 4):128 * (j % 4) + 128],
                        in_=b2[:, h, j, :], identity=ident[:, :])
                    if j % 4 == 3:
                        g = j // 4
                        nc.scalar.activation(
                            out=eeT[h][:, 4 * g:4 * g + 4, :],
                            in_=tph[h][:, :],
                            func=mybir.ActivationFunctionType.Exp)

            # ---------- zT, rzT, x2t ----------
            rz = []
            with nc.allow_low_precision("softmax denom in bf16 is plenty"):
                for h in range(2):
                    t16 = work.tile([128, 16, B], bf16, tag="zt16")
                    nc.vector.tensor_tensor(
                        out=t16[:, :, :], in0=eeT[h][:, 0:16, :],
                        in1=eeT[h][:, 16:32, :], op=mybir.AluOpType.add)
                    t8 = work.tile([128, 8, B], bf16, tag="zt8")
                    nc.vector.tensor_tensor(
                        out=t8[:, :, :], in0=t16[:, 0:8, :],
                        in1=t16[:, 8:16, :], op=mybir.AluOpType.add)
                    t4 = work.tile([128, 4, B], bf16, tag="zt4")
                    nc.vector.tensor_tensor(
                        out=t4[:, :, :], in0=t8[:, 0:4, :], in1=t8[:, 4:8, :],
                        op=mybir.AluOpType.add)
                    t2 = work.tile([128, 2, B], bf16, tag="zt2")
                    nc.vector.tensor_tensor(
                        out=t2[:, :, :], in0=t4[:, 0:2, :], in1=t4[:, 2:4, :],
                        op=mybir.AluOpType.add)
                    z1 = work.tile([128, B], bf16, tag="zt1")
                    nc.vector.tensor_tensor(
                        out=z1[:, :], in0=t2[:, 0, :], in1=t2[:, 1, :],
                        op=mybir.AluOpType.add)
                    rzh = consts.tile([128, B], bf16, tag=f"rz{h}",
                                      name=f"rz{h}")
                    nc.vector.reciprocal(out=rzh[:, :], in_=z1[:, :])
                    rz.append(rzh)
            x2t = consts.tile([128, 16, B], bf16, tag="x2t")
            for h in range(2):
                nc.vector.tensor_tensor(
                    out=_strided(x2t[:, h, 0:1], [[256, 8], [1, B]]),
                    in0=_strided(xt[:, h, 0:1], [[256, 8], [1, B]]),
                    in1=_bc_mid(rz[h][:, :], 8), op=mybir.AluOpType.mult)

            # ---------- pass 2b: yT + s2 GEMM ----------
            ps2 = pacc.tile([B, JD], f32, tag="ps2", name="ps2")
            for kt in range(16):
                h = kt % 2
                yt = ypool.tile([128, J, B], bf16, tag="yt")
                for jh in range(2):
                    eng = (nc.vector if YT_ENG[kt * 2 + jh] == 'dve'
                           else nc.gpsimd)
                    eng.tensor_tensor(
                        out=yt[:, 16 * jh:16 * jh + 16, :],
                        in0=_bc_mid(x2t[:, kt, :], 16),
                        in1=eeT[h][:, 16 * jh:16 * jh + 16, :],
                        op=mybir.AluOpType.mult)
                # ps2 is a single psum bank; one accumulation group for all
                # (kt, j): start marks the whole bank pending-zero, each
                # column range is first written (zeroed) then accumulated.
                for j in range(J):
                    nc.tensor.matmul(
                        out=ps2[:, 16 * j:16 * j + 16],
                        lhsT=yt[:, j, :], rhs=ws[:, j, kt, :],
                        start=(kt == 0 and j == 0),
                        stop=(kt == 15 and j == J - 1))

            s2c = work.tile([B, JD], f32, tag="s2c")
            nc.vector.tensor_copy(s2c[:, :], ps2[:, :])
            nc.sync.dma_start(out=cc2_in.ap(), in_=s2c[:, :])
            allreduce(cc2_in, cc2_out)
            s2 = consts.tile([B, JD], f32, tag="s2")
            nc.sync.dma_start(out=s2[:, :], in_=cc2_out.ap())

            # ---------- v2 = squash(s2) -> output ----------
            v2 = consts.tile([B, JD], f32, tag="v2")
            _squash(nc, work, s2, v2)
            nc.sync.dma_start(out=yout.ap(), in_=v2[:, :])

    nc.compile()
    return nc


_NC_CACHE = None


def _get_nc():
    global _NC_CACHE
    if _NC_CACHE is None:
        _NC_CACHE = build_nc()
    return _NC_CACHE


def _prep_inputs(x, W):
    """Pure layout transform + bf16 cast: returns per-core input dicts."""
    bf = ml_dtypes.bfloat16
    x = np.asarray(x, dtype=np.float32)
    W = np.asarray(W, dtype=np.float32)
    xc = x.reshape(B, NC_, IS, E)             # [b, c, il, e]
    Wc = W[0].reshape(NC_, IS, J, D, E)       # [c, il, j, d, e]
    ident = np.eye(128, dtype=np.float32)
    outs = []
    for c in range(NC_):
        xv = xc[:, c]                          # [B, 256, 8]
        Wv = Wc[c]                             # [256, J, D, E]
        # xt[lane, kt=(e,half), b] = x[b, half*128+lane, e]
        a = xv.transpose(1, 2, 0).reshape(2, 128, E, B)   # [half, lane, e, b]
        xt = np.ascontiguousarray(a.transpose(1, 2, 0, 3)).reshape(128, 16, B)
        # xb[b, (il, e)]
        xb = np.ascontiguousarray(xv.reshape(B, IS * E))
        # w1[lane, kt, (j,d)] = W[il(kt,lane), j, d, e(kt)] / 32
        a = Wv.reshape(2, 128, J, D, E).transpose(1, 4, 0, 2, 3)
        w1 = np.ascontiguousarray(a).reshape(128, 16, JD) * (1.0 / J)
        # wa[(j%4)*32 + r, j//4, (il, e)]: r<16 -> d=r, r>=16 -> 0
        bmat = Wv.transpose(1, 2, 0, 3).reshape(J // 4, 4, D, IS, E)
        arr = np.zeros((4, 32, J // 4, IS, E), dtype=np.float32)
        arr[:, :16] = bmat.transpose(1, 2, 0, 3, 4)   # [s, d, t, il, e]
        wa = arr.reshape(128, J // 4, IS * E)
        # ws[lane, j, kt, d] = W[il(kt,lane), j, d, e(kt)]
        a = Wv.reshape(2, 128, J, D, E).transpose(1, 2, 4, 0, 3)
        ws = np.ascontiguousarray(a).reshape(128, J, 16, D)
        outs.append({
            "xt": xt.astype(bf), "xb": xb.astype(bf), "w1": w1.astype(bf),
            "wa": wa.astype(bf), "ws": ws.astype(bf),
            "ident": ident,
        })
    return outs


def run(x, W, trace=False, **kw):
    in_maps = _prep_inputs(x, W)
    nc = _get_nc()
    res = run_bass_kernel_spmd(nc, in_maps, core_ids=list(range(NC_)),
                               trace=trace, **kw)
    out = res.results[0]["yout"].reshape(B, J, D).astype(np.float32)
    return out, res


def kernel(x, W):
    out, _ = run(x, W)
    return out
